# revision 9
# baseline (speedup 1.0000x reference)
"""Deep-hedging GRU kernel for 8 Trainium2 NeuronCores.

Data-parallel over n_sim: 16384 paths -> 2048 per core.  Per-core layout is
feature-major: hidden state h is [H=128 partitions, 2048 paths free], the 63
time steps run as a fully unrolled recurrence.

Per step, per 512-path tile j (4 tiles):
  psum_rzin[128,1536] <- W_hh_r@h (+) W_gin_r@gin | z likewise | i_n (gin only)
  psum_hn[128,512]    <- W_hh_n@h
  rz = sigmoid(psum_rzin[:, :1024])            (ACT, one pass for r||z)
  t1 = (hn + b_hh_n)*r                         (DVE scalar_tensor_tensor)
  t2 = t1 + i_n ; n = tanh(t2)                 (DVE + ACT)
  h' = n + z*(h - n)                           (3 DVE tensor_tensor)
  d_psum[32j:32j+8, :] <- W_out.T-col-tiled matmul of h'
then pos update packed over all 4 tiles in [128,512]:
  q = (d + b_out) + pos ; pos' = clip(q, -1, 1) ; dout = pos' - pos
gin carries [pos(8); x(16); ones(1)] rows per 32-row group so gate matmuls
pick up x, pos and both biases in one K=25 accumulation.
"""

import numpy as np

import concourse.bass as bass
import concourse.tile as tile
from concourse import bacc, mybir
from concourse.bass_utils import run_bass_kernel_spmd

F32 = mybir.dt.float32
AF = mybir.ActivationFunctionType
OP = mybir.AluOpType

N_CORES = 8
NSIM, NSTEP, IND = 16384, 64, 16
H, O = 128, 8
T = NSTEP - 1            # 63 recurrence steps
P = NSIM // N_CORES      # 2048 paths per core
NT = 4                   # path tiles per core
TN = P // NT             # 512 paths per tile
CAP = 1.0

_cached = {}
_last_results = None


def _build_program():
    nc = bacc.Bacc("TRN2", target_bir_lowering=False, debug=False)

    xp = nc.dram_tensor("xp", [T, NT, 24, TN], F32, kind="ExternalInput")
    wgin = nc.dram_tensor("wgin", [128, 3 * H], F32, kind="ExternalInput")
    whht = nc.dram_tensor("whht", [H, 3 * H], F32, kind="ExternalInput")
    woutt = nc.dram_tensor("woutt", [H, 32], F32, kind="ExternalInput")
    bhn = nc.dram_tensor("bhn", [H, 1], F32, kind="ExternalInput")
    boutp = nc.dram_tensor("boutp", [128, 1], F32, kind="ExternalInput")
    y = nc.dram_tensor("y", [T, NT, O, TN], F32, kind="ExternalOutput")

    with tile.TileContext(nc) as tc:
        from contextlib import ExitStack

        with ExitStack() as ctx:
            persist = ctx.enter_context(tc.tile_pool(name="persist", bufs=1))
            rzin_pool = ctx.enter_context(
                tc.tile_pool(name="rzin", bufs=2, space="PSUM")
            )
            small_ps = ctx.enter_context(
                tc.tile_pool(name="smallps", bufs=2, space="PSUM")
            )
            sb = ctx.enter_context(tc.tile_pool(name="work", bufs=3))

            # --- persistent tiles -------------------------------------------------
            w_gin = persist.tile([128, 3 * H], F32, tag="w_gin")
            w_hht = persist.tile([H, 3 * H], F32, tag="w_hht")
            w_outt = persist.tile([H, 32], F32, tag="w_outt")
            b_hn = persist.tile([H, 1], F32, tag="b_hn")
            b_outp = persist.tile([128, 1], F32, tag="b_outp")
            nc.sync.dma_start(w_gin[:], wgin.ap())
            nc.sync.dma_start(w_hht[:], whht.ap())
            nc.sync.dma_start(w_outt[:], woutt.ap())
            nc.sync.dma_start(b_hn[:], bhn.ap())
            nc.sync.dma_start(b_outp[:], boutp.ap())

            h_buf = [persist.tile([H, P], F32, tag=f"h{i}", name=f"h{i}") for i in range(2)]
            gin_buf = [persist.tile([128, TN], F32, tag=f"gin{i}", name=f"gin{i}") for i in range(2)]
            pos_buf = [persist.tile([128, TN], F32, tag=f"pos{i}", name=f"pos{i}") for i in range(2)]

            nc.gpsimd.memset(h_buf[0][:], 0.0)
            nc.gpsimd.memset(pos_buf[0][:], 0.0)
            for j in range(NT):
                nc.vector.memset(gin_buf[0][32 * j : 32 * j + 8, :], 0.0)
            # x rows of gin for step 0
            for j in range(NT):
                nc.sync.dma_start(
                    gin_buf[0][32 * j + 8 : 32 * (j + 1), :], xp.ap()[0, j]
                )

            # --- recurrence -------------------------------------------------------
            for t in range(T):
                gc = gin_buf[t % 2]
                gn = gin_buf[(t + 1) % 2]
                hc = h_buf[t % 2]
                hn_buf = h_buf[(t + 1) % 2]
                pc = pos_buf[t % 2]
                pn = pos_buf[(t + 1) % 2]

                # prefetch next step's x rows
                if t + 1 < T:
                    for j in range(NT):
                        nc.sync.dma_start(
                            gn[32 * j + 8 : 32 * (j + 1), :], xp.ap()[t + 1, j]
                        )

                d_ps = small_ps.tile([128, TN], F32, tag="small")
                for j in range(NT):
                    cols = slice(TN * j, TN * (j + 1))
                    gslice = gc[32 * j : 32 * j + 25, :]
                    rzin = rzin_pool.tile([128, 3 * TN], F32, tag="rzin")
                    hn_ps = small_ps.tile([128, TN], F32, tag="small")

                    # r gate: W_hh_r @ h  (+)  W_gin_r @ gin
                    nc.tensor.matmul(
                        rzin[:, 0:TN], w_hht[:, 0:H], hc[:, cols],
                        start=True, stop=False,
                    )
                    nc.tensor.matmul(
                        rzin[:, 0:TN], w_gin[32 * j : 32 * j + 25, 0:H], gslice,
                        start=False, stop=True, tile_position=(32 * j, 0),
                    )
                    # z gate
                    nc.tensor.matmul(
                        rzin[:, TN : 2 * TN], w_hht[:, H : 2 * H], hc[:, cols],
                        start=True, stop=False,
                    )
                    nc.tensor.matmul(
                        rzin[:, TN : 2 * TN],
                        w_gin[32 * j : 32 * j + 25, H : 2 * H], gslice,
                        start=False, stop=True, tile_position=(32 * j, 0),
                    )
                    # i_n (gin only)
                    nc.tensor.matmul(
                        rzin[:, 2 * TN : 3 * TN],
                        w_gin[32 * j : 32 * j + 25, 2 * H : 3 * H], gslice,
                        start=True, stop=True, tile_position=(32 * j, 0),
                    )
                    # h_n (hh only)
                    nc.tensor.matmul(
                        hn_ps[:], w_hht[:, 2 * H : 3 * H], hc[:, cols],
                        start=True, stop=True,
                    )

                    rz = sb.tile([128, 2 * TN], F32, tag="rz")
                    nc.scalar.activation(rz[:], rzin[:, 0 : 2 * TN], AF.Sigmoid)

                    t1 = sb.tile([128, TN], F32, tag="t1")
                    nc.vector.scalar_tensor_tensor(
                        t1[:], hn_ps[:], b_hn[:], rz[:, 0:TN],
                        op0=OP.add, op1=OP.mult,
                    )
                    t2 = sb.tile([128, TN], F32, tag="t2")
                    nc.vector.tensor_add(t2[:], t1[:], rzin[:, 2 * TN : 3 * TN])
                    n_sb = sb.tile([128, TN], F32, tag="n")
                    nc.scalar.activation(n_sb[:], t2[:], AF.Tanh)

                    t3 = sb.tile([128, TN], F32, tag="t3")
                    nc.vector.tensor_sub(t3[:], hc[:, cols], n_sb[:])
                    t4 = sb.tile([128, TN], F32, tag="t4")
                    nc.vector.tensor_mul(t4[:], rz[:, TN : 2 * TN], t3[:])
                    nc.vector.tensor_add(hn_buf[:, cols], n_sb[:], t4[:])

                    # d tile -> psum rows 32j..32j+8 (col-group j)
                    nc.tensor.matmul(
                        d_ps[32 * j : 32 * (j + 1), :], w_outt[:], hn_buf[:, cols],
                        start=True, stop=True, tile_position=(0, 32 * j),
                    )

                # pos pipeline, packed over all 4 tiles
                q = sb.tile([128, TN], F32, tag="q")
                nc.vector.scalar_tensor_tensor(
                    q[:], d_ps[:], b_outp[:], pc[:], op0=OP.add, op1=OP.add
                )
                nc.vector.tensor_scalar(
                    pn[:], q[:], -CAP, CAP, op0=OP.max, op1=OP.min
                )
                dout = sb.tile([128, TN], F32, tag="dout")
                nc.vector.tensor_sub(dout[:], pn[:], pc[:])

                for j in range(NT):
                    nc.sync.dma_start(
                        y.ap()[t, j], dout[32 * j : 32 * j + O, :]
                    )
                if t + 1 < T:
                    for j in range(NT):
                        nc.sync.dma_start(
                            gn[32 * j : 32 * j + 8, :], pn[32 * j : 32 * j + 8, :]
                        )
    nc.compile()
    return nc


def _prep_core_inputs(X, W_ih, W_hh, b_ih, b_hh, W_out, b_out):
    """Host-side prep: per-core feature-major X + packed weight operands."""
    X = np.asarray(X, np.float32)
    W_ih = np.asarray(W_ih, np.float32)
    W_hh = np.asarray(W_hh, np.float32)
    b_ih = np.asarray(b_ih, np.float32)
    b_hh = np.asarray(b_hh, np.float32)
    W_out = np.asarray(W_out, np.float32)
    b_out = np.asarray(b_out, np.float32)

    base = np.zeros((32, 3 * H), np.float32)
    base[0:8] = W_ih[:, IND : IND + O].T          # pos rows
    base[8:24] = W_ih[:, 0:IND].T                 # x rows
    bias = np.concatenate(
        [b_ih[0:H] + b_hh[0:H], b_ih[H : 2 * H] + b_hh[H : 2 * H], b_ih[2 * H :]]
    )
    base[24] = bias                               # ones row
    wgin = np.ascontiguousarray(np.tile(base, (NT, 1)))

    whht = np.ascontiguousarray(W_hh.T)           # [128, 384]
    woutt = np.zeros((H, 32), np.float32)
    woutt[:, :O] = W_out.T
    bhn = np.ascontiguousarray(b_hh[2 * H :].reshape(H, 1))
    brow = np.zeros(32, np.float32)
    brow[:O] = b_out
    boutp = np.ascontiguousarray(np.tile(brow, NT).reshape(128, 1))

    in_maps = []
    for c in range(N_CORES):
        Xc = X[c * P : (c + 1) * P, :T, :]        # [2048, 63, 16]
        xp = np.zeros((T, NT, 24, TN), np.float32)
        xp[:, :, :IND, :] = Xc.reshape(NT, TN, T, IND).transpose(2, 0, 3, 1)
        xp[:, :, IND, :] = 1.0
        in_maps.append(
            {
                "xp": xp,
                "wgin": wgin,
                "whht": whht,
                "woutt": woutt,
                "bhn": bhn,
                "boutp": boutp,
            }
        )
    return in_maps


def kernel(X, W_ih, W_hh, b_ih, b_hh, W_out, b_out):
    global _last_results
    if "nc" not in _cached:
        _cached["nc"] = _build_program()
    nc = _cached["nc"]

    in_maps = _prep_core_inputs(X, W_ih, W_hh, b_ih, b_hh, W_out, b_out)
    res = run_bass_kernel_spmd(nc, in_maps, core_ids=list(range(N_CORES)))
    _last_results = res

    out = np.empty((NSIM, T, O), np.float32)
    for c in range(N_CORES):
        yc = res.results[c]["y"]                  # [63, 4, 8, 512]
        out[c * P : (c + 1) * P] = yc.transpose(1, 3, 0, 2).reshape(P, T, O)
    return out
